# revision 1
# baseline (speedup 1.0000x reference)
"""GroupedQueryAttention Trainium2 kernel (8-core SPMD).

Reference op: RMSNorm -> in-proj (q/k/v) -> RoPE -> causal GQA attention
-> out-proj -> residual.  b=2, s=2048, d_model=2048, 32 q-heads / 8 KV
groups, head dim 64, fp32.

Sharding: core c handles batch b = c//4 and KV groups (2j, 2j+1), j = c%4
(data parallel over batch x tensor parallel over KV groups, Megatron
style).  Each core computes the full in-projection restricted to its 8
heads' channels, attention for its 8 heads, and a partial out-projection
(row-parallel).  The host sums the 4 partials per batch and adds the
residual.

Device-side layout notes:
  * Everything is kept "transposed" ([feature, token]) so all matmuls
    contract over the partition dim.  x.T is produced on the host.
  * rms_w is folded into w_in columns; 1/sqrt(d_qk) is folded into the
    q rows of w_in; the per-token 1/rms scale is applied after the
    in-projection (it commutes with the linear projection).
  * matmuls run with float32r (fp22-truncated fp32) operands: full PE
    speed at ~1e-4 relative accuracy.
  * softmax skips the max-subtraction (logits are O(6) here, exp is safe
    in fp32) and gets its denominator for free from a ones-column
    appended to V (output row 64 of the AV matmul).
  * QK^T for the two group-heads of a pair runs as two concurrent
    K=64 matmuls via PE row-tiling (tile_position (0,0)/(64,0)).
"""

import numpy as np
from contextlib import ExitStack

import concourse.bass as bass
from concourse import bacc as _bacc
import concourse.mybir as mybir
import concourse.tile as tile
from concourse.bass import ts

import os
f32 = mybir.dt.float32
f32r = mybir.dt.float32r
f16 = mybir.dt.float16
MDT = {"f32r": f32r, "f16": f16, "bf16": mybir.dt.bfloat16}[os.environ.get("GQA_MM_DT", "f16")]
try:
    import ml_dtypes
    _BF16_NP = ml_dtypes.bfloat16
except ImportError:
    _BF16_NP = None
MDT_NP = {f32r: np.float32, f16: np.float16, mybir.dt.bfloat16: _BF16_NP}[MDT]
AF = mybir.ActivationFunctionType
ALU = mybir.AluOpType

D = 2048          # model dim
CH = 768          # per-core in-proj channels (8 q heads + 2 k + 2 v)
TOKC = 512        # token chunk
NKT = D // 128    # 16 k-tiles over model dim
RMS_EPS = 1e-6
ROPE_THETA = 10000.0
NCORES = 8


def build_program(S=2048):
    NCH = S // TOKC          # token chunks
    NSK = S // 128           # sk tiles
    nc = _bacc.Bacc(None)

    xT_d = nc.dram_tensor("xT", [D, S], MDT, kind="ExternalInput")
    w_inT_d = nc.dram_tensor("w_inT", [D, CH], MDT, kind="ExternalInput")
    w_outT_d = nc.dram_tensor("w_outT", [512, D], MDT, kind="ExternalInput")
    cos_d = nc.dram_tensor("cos_t", [128, S], f32, kind="ExternalInput")
    sin_d = nc.dram_tensor("sin_t", [128, S], f32, kind="ExternalInput")
    tri_d = nc.dram_tensor("tri", [128, 128], MDT, kind="ExternalInput")
    id64_d = nc.dram_tensor("id64", [64, 64], MDT, kind="ExternalInput")
    oner_d = nc.dram_tensor("oner", [1], MDT, kind="ExternalInput")
    eps_d = nc.dram_tensor("epsc", [1], f32, kind="ExternalInput")
    yT_d = nc.dram_tensor("yT", [D, S], f32, kind="ExternalOutput")

    with tile.TileContext(nc) as tc, ExitStack() as ctx:
        sb = ctx.enter_context(tc.tile_pool(name="sb", bufs=1))
        sbs = ctx.enter_context(tc.tile_pool(name="sbs", bufs=2))
        dramp = ctx.enter_context(tc.tile_pool(name="dram", bufs=1, space="DRAM"))

        # persistent SBUF
        w_in_sb = sb.tile([128, NKT, CH], MDT, name="w_in_sb")
        qkv = sb.tile([128, 6, S], MDT, name="qkv")        # ch tiles 0-3 q pairs, 4 k, 5 v
        oT = sb.tile([128, 4, S], MDT, name="oT")
        vA = sb.tile([128, NSK, 65], MDT, name="vA")       # V + ones col, group 0
        vB = sb.tile([128, NSK, 65], MDT, name="vB")       # group 1
        tri_sb = sb.tile([128, 128], MDT, name="tri_sb")
        id128_sb = sb.tile([128, 64], MDT, name="id128_sb")
        ones_sb = sb.tile([128, 1], MDT, name="ones_sb")

        nrm_dr = dramp.tile([NCH, TOKC], f32, name="nrm_dr")
        db_dr = dramp.tile([NCH, 4, 2, TOKC], f32, name="db_dr")
        nrm2_dr = dramp.tile([NCH, TOKC], f32, name="nrm2_dr")
        db2_dr = dramp.tile([NCH, 4, 2, TOKC], f32, name="db2_dr")

        w_inT_v = w_inT_d.rearrange("(o p) c -> p o c", p=128)
        for kt in range(NKT):
            nc.sync.dma_start(w_in_sb[:, kt, :], w_inT_v[:, kt, :])
        nc.sync.dma_start(tri_sb[:], tri_d[:])
        nc.sync.dma_start(id128_sb[0:64, :], id64_d[:])
        nc.sync.dma_start(id128_sb[64:128, :], id64_d[:])
        eps_sb = sb.tile([1, 1], f32, name="eps_sb")
        nc.sync.dma_start(ones_sb[:], oner_d[None, :].to_broadcast((128, 1)))
        nc.sync.dma_start(vA[:, :, 64:65], oner_d[None, None, :].to_broadcast((128, NSK, 1)))
        nc.sync.dma_start(vB[:, :, 64:65], oner_d[None, None, :].to_broadcast((128, NSK, 1)))
        nc.sync.dma_start(eps_sb[:], eps_d[None, :])

        # One PSUM pool for everything; tags sized so in-projection of chunk
        # c+1 (pure PE work) overlaps attention of chunk c (ACT-exp heavy):
        # ip(2) + ss(1) + qk(2) + av(2) + op(1) = 8 banks.
        with tc.tile_pool(name="ps", bufs=1, space="PSUM") as ps:

            def emit_prelude(c):
                cs = slice(c * TOKC, (c + 1) * TOKC)
                # x tiles for this chunk stay resident through the m-loop
                xts = []
                for kt in range(NKT):
                    xt = sbs.tile([128, TOKC], MDT, tag="xt", bufs=20,
                                  name=f"xt_{c}_{kt}")
                    nc.sync.dma_start(xt[:], xT_d[ts(kt, 128), cs])
                    xts.append(xt)
                # sum of squares first: it gates the norm chain + rope
                ss = ps.tile([1, TOKC], f32, tag="ss", bufs=1, name=f"ss_{c}")
                for kt in range(NKT):
                    xsq = sbs.tile([128, TOKC], MDT, tag="xsq", bufs=2,
                                   name=f"xsq_{c}_{kt}")
                    nc.vector.tensor_tensor(xsq[:], xts[kt][:], xts[kt][:], ALU.mult)
                    nc.tensor.matmul(ss[:], ones_sb[:], xsq[:],
                                     start=(kt == 0), stop=(kt == NKT - 1))
                # norm chain: inv_rms = 1/sqrt(ss/D + eps); reciprocal is done in
                # a [128, 4] token-partition layout (a [1, 512] reciprocal costs
                # 3.3us on one DVE lane)
                sqm = sbs.tile([1, TOKC], f32, tag="sqm", bufs=1, name=f"sqm_{c}")
                nc.scalar.activation(sqm[:], ss[:], AF.Sqrt,
                                     bias=eps_sb[:], scale=1.0 / D)
                nc.sync.dma_start(nrm_dr[c][None, :], sqm[:])
                srT = sbs.tile([128, TOKC // 128], f32, tag="srT", bufs=2,
                               name=f"srT_{c}")
                nc.sync.dma_start(srT[:], nrm_dr[c].rearrange("(a p) -> p a", p=128))
                invT = sbs.tile([128, TOKC // 128], f32, tag="invT", bufs=2,
                                name=f"invT_{c}")
                nc.vector.reciprocal(invT[:], srT[:])
                nc.sync.dma_start(nrm2_dr[c].rearrange("(a p) -> p a", p=128), invT[:])
                inv128 = sbs.tile([128, TOKC], f32, tag="inv128", bufs=2,
                                  name=f"inv128_{c}")
                nc.sync.dma_start(inv128[:],
                                  nrm2_dr[c][None, :].to_broadcast((128, TOKC)))
                # rope tables scaled by inv_rms
                cos_c = sbs.tile([128, TOKC], f32, tag="cos_c", bufs=2, name=f"cos_c_{c}")
                nc.sync.dma_start(cos_c[:], cos_d[:, cs])
                sin_c = sbs.tile([128, TOKC], f32, tag="sin_c", bufs=2, name=f"sin_c_{c}")
                nc.sync.dma_start(sin_c[:], sin_d[:, cs])
                cosi = sbs.tile([128, TOKC], f32, tag="cosi", bufs=2, name=f"cosi_{c}")
                nc.vector.tensor_tensor(cosi[:], cos_c[:], inv128[:], ALU.mult)
                sini = sbs.tile([128, TOKC], f32, tag="sini", bufs=2, name=f"sini_{c}")
                nc.vector.tensor_tensor(sini[:], sin_c[:], inv128[:], ALU.mult)
                return xts, cosi, sini, invT

            def emit_inproj_m(c, m, state):
                cs = slice(c * TOKC, (c + 1) * TOKC)
                xts, cosi, sini, invT = state
                if True:
                    ip = ps.tile([128, TOKC], f32, tag="ip", bufs=2,
                                 name=f"ip{m}_{c}")
                    for kt in range(NKT):
                        nc.tensor.matmul(ip[:], w_in_sb[:, kt, ts(m, 128)], xts[kt][:],
                                         start=(kt == 0), stop=(kt == NKT - 1))
                    nc.vector.tensor_copy(qkv[:, m, cs], ip[:])
                    if m < 5:
                        # rope in place, inv_rms folded into the tables.
                        # tmp[dst] = x[src]*s2[src]: inputs share a base
                        # partition (walrus requires it), output is shifted.
                        tmp = sbs.tile([128, TOKC], f32, tag="rtmp", bufs=2,
                                       name=f"rtmp_{c}_{m}")
                        for dst, src in ((0, 32), (32, 0), (64, 96), (96, 64)):
                            nc.vector.tensor_tensor(
                                tmp[dst:dst + 32, :],
                                qkv[src:src + 32, m, cs],
                                sini[src:src + 32, :],
                                ALU.mult,
                            )
                        nc.vector.tensor_tensor(qkv[:, m, cs], qkv[:, m, cs],
                                                cosi[:], ALU.mult)
                        nc.vector.tensor_tensor(qkv[:, m, cs], qkv[:, m, cs],
                                                tmp[:], ALU.add)
                    else:
                        # V: transpose to [token, dv] (+ inv_rms per-token scale)
                        for tl in range(TOKC // 128):
                            t = c * (TOKC // 128) + tl
                            vtt = sbs.tile([128, 128], MDT, tag="vtt", bufs=2,
                                           name=f"vtt_{t}")
                            nc.sync.dma_start(vtt[:], qkv[:, 5, ts(t, 128)],
                                              transpose=True)
                            nc.scalar.activation(vA[:, t, 0:64], vtt[:, 0:64],
                                                 AF.Copy, scale=invT[:, tl:tl + 1])
                            nc.scalar.activation(vB[:, t, 0:64], vtt[:, 64:128],
                                                 AF.Copy, scale=invT[:, tl:tl + 1])

            def emit_attn_pair(c, p):
                cs = slice(c * TOKC, (c + 1) * TOKC)
                n_t = 4 * (c + 1)
                if True:
                    avA = ps.tile([65, TOKC], f32, tag="av", bufs=2,
                                  name=f"avA_{c}_{p}")
                    avB = ps.tile([65, TOKC], f32, tag="av", bufs=2,
                                  name=f"avB_{c}_{p}")
                    for t in range(n_t):
                        j0 = max(0, t - 4 * c) * 128
                        qk = ps.tile([128, 2, TOKC], f32, tag="qk", bufs=1,
                                     name=f"qk_{c}_{p}_{t}")
                        # the pair's two heads: row-tiled concurrent K=64 matmuls
                        nc.tensor.matmul(
                            qk[:, 0, j0:],
                            qkv[0:64, 4, ts(t, 128)],
                            qkv[0:64, p, c * TOKC + j0:(c + 1) * TOKC],
                            start=True, stop=True,
                        )
                        nc.tensor.matmul(
                            qk[:, 1, j0:],
                            qkv[64:128, 4, ts(t, 128)],
                            qkv[64:128, p, c * TOKC + j0:(c + 1) * TOKC],
                            start=True, stop=True,
                        )
                        e = sbs.tile([128, 2, TOKC], MDT, tag="e", bufs=3,
                                     name=f"e_{c}_{p}_{t}")
                        nc.scalar.activation(e[:, :, j0:], qk[:, :, j0:], AF.Exp)
                        if t >= 4 * c:  # diagonal tile: causal mask
                            for h in (0, 1):
                                nc.vector.tensor_tensor(
                                    e[:, h, j0:j0 + 128],
                                    e[:, h, j0:j0 + 128],
                                    tri_sb[:],
                                    ALU.mult,
                                )
                        nc.tensor.matmul(avA[:, j0:], vA[:, t, :], e[:, 0, j0:],
                                         start=(t == 0), stop=(t == n_t - 1))
                        nc.tensor.matmul(avB[:, j0:], vB[:, t, :], e[:, 1, j0:],
                                         start=(t == 0), stop=(t == n_t - 1))
                    # softmax denominators: row 64 of each AV psum.  Reciprocal
                    # runs in a [128, 2, 4] layout via a DRAM bounce.
                    d2 = sbs.tile([1, 2 * TOKC], f32, tag="d2", bufs=2,
                                  name=f"d2_{c}_{p}")
                    nc.scalar.copy(d2[:, 0:TOKC], avA[64:65, :])
                    nc.scalar.copy(d2[:, TOKC:], avB[64:65, :])
                    nc.sync.dma_start(
                        db_dr[c, p].rearrange("g t -> (g t)")[None, :], d2[:])
                    dT = sbs.tile([128, 2, TOKC // 128], f32, tag="dT", bufs=2,
                                  name=f"dT_{c}_{p}")
                    nc.sync.dma_start(
                        dT[:], db_dr[c, p].rearrange("g (a p) -> p g a", p=128))
                    dTi = sbs.tile([128, 2, TOKC // 128], f32, tag="dTi", bufs=2,
                                   name=f"dTi_{c}_{p}")
                    nc.vector.reciprocal(dTi[:], dT[:])
                    nc.sync.dma_start(
                        db2_dr[c, p].rearrange("g (a p) -> p g a", p=128), dTi[:])
                    dbA = sbs.tile([64, TOKC], f32, tag="dbA", bufs=2,
                                   name=f"dbA_{c}_{p}")
                    nc.sync.dma_start(
                        dbA[:], db2_dr[c, p, 0][None, :].to_broadcast((64, TOKC)))
                    dbB = sbs.tile([64, TOKC], f32, tag="dbB", bufs=2,
                                   name=f"dbB_{c}_{p}")
                    nc.sync.dma_start(
                        dbB[:], db2_dr[c, p, 1][None, :].to_broadcast((64, TOKC)))
                    nc.vector.tensor_tensor(oT[0:64, p, cs], avA[0:64, :],
                                            dbA[:], ALU.mult)
                    nc.vector.tensor_tensor(oT[64:128, p, cs], avB[0:64, :],
                                            dbB[:], ALU.mult)

            def emit_outproj_part(c, ms):
                cs = slice(c * TOKC, (c + 1) * TOKC)
                for m in ms:
                    wos = []
                    for kt in range(4):
                        wo = sbs.tile([128, 128], MDT, tag="wo", bufs=8,
                                      name=f"wo_{c}_{m}_{kt}")
                        nc.sync.dma_start(wo[:], w_outT_d[ts(kt, 128), ts(m, 128)])
                        wos.append(wo)
                    op = ps.tile([128, TOKC], f32, tag="op", bufs=1,
                                 name=f"op_{c}_{m}")
                    for kt in range(4):
                        nc.tensor.matmul(op[:], wos[kt][:], oT[:, kt, cs],
                                         start=(kt == 0), stop=(kt == 3))
                    yt = sbs.tile([128, TOKC], f32, tag="yt", bufs=2,
                                  name=f"yt_{c}_{m}")
                    nc.vector.tensor_copy(yt[:], op[:])
                    nc.sync.dma_start(yT_d[ts(m, 128), cs], yt[:])

            for c in range(NCH):
                st = emit_prelude(c)
                for m in range(6):
                    emit_inproj_m(c, m, st)
                    if c > 0 and m < 4:
                        emit_attn_pair(c - 1, m)
                if c > 0:
                    emit_outproj_part(c - 1, range(16))
            for p in range(4):
                emit_attn_pair(NCH - 1, p)
            emit_outproj_part(NCH - 1, range(16))

    nc.finalize()
    return nc


# ------------------------------- host side ----------------------------------

def _rope_tables(S):
    inv_freq = ROPE_THETA ** (-np.arange(0, 64, 2, dtype=np.float64) / 64.0)  # [32]
    ang = np.arange(S, dtype=np.float64)[:, None] * inv_freq[None, :]          # [S, 32]
    cosb = np.cos(ang).T.astype(np.float32)   # [32, S]
    sinb = np.sin(ang).T.astype(np.float32)
    cos128 = np.tile(cosb, (4, 1))                                             # [128, S]
    sin128 = np.concatenate([sinb, -sinb, sinb, -sinb], axis=0)                # [128, S]
    return np.ascontiguousarray(cos128), np.ascontiguousarray(sin128)


def host_prepare(x, w_in, w_out, rms_w):
    """Build the 8 per-core input maps."""
    S = x.shape[1]
    x = np.asarray(x, dtype=np.float32)
    w_eff = np.asarray(w_in, dtype=np.float32) * np.asarray(rms_w, np.float32)[None, :]
    w_out = np.asarray(w_out, dtype=np.float32)
    cos128, sin128 = _rope_tables(S)
    tri = np.ascontiguousarray(np.triu(np.ones((128, 128), dtype=np.float32)))
    id64 = np.eye(64, dtype=np.float32)
    qscale = np.float32(64 ** -0.5)

    in_maps = []
    for core in range(NCORES):
        b, j = divmod(core, 4)
        g0, g1 = 2 * j, 2 * j + 1
        rows = []
        for p in range(4):
            for g in (g0, g1):
                rows.extend(range((g * 4 + p) * 64, (g * 4 + p) * 64 + 64))
        for g in (g0, g1):
            rows.extend(range(2048 + g * 64, 2048 + g * 64 + 64))
        for g in (g0, g1):
            rows.extend(range(2560 + g * 64, 2560 + g * 64 + 64))
        w_slice = w_eff[rows, :].copy()
        w_slice[:512, :] *= qscale
        cols = []
        for p in range(4):
            for g in (g0, g1):
                cols.extend(range((g * 4 + p) * 64, (g * 4 + p) * 64 + 64))
        in_maps.append({
            "xT": np.ascontiguousarray(x[b].T).astype(MDT_NP),
            "w_inT": np.ascontiguousarray(w_slice.T).astype(MDT_NP),
            "w_outT": np.ascontiguousarray(w_out[:, cols].T).astype(MDT_NP),
            "cos_t": cos128,
            "sin_t": sin128,
            "tri": tri.astype(MDT_NP),
            "id64": id64.astype(MDT_NP),
            "oner": np.ones(1, dtype=MDT_NP),
            "epsc": np.full(1, RMS_EPS, dtype=np.float32),
        })
    return in_maps


def assemble(x, results):
    x = np.asarray(x, dtype=np.float32)
    b0 = results[0]["yT"] + results[1]["yT"] + results[2]["yT"] + results[3]["yT"]
    b1 = results[4]["yT"] + results[5]["yT"] + results[6]["yT"] + results[7]["yT"]
    out = np.empty_like(x)
    out[0] = x[0] + b0.T
    out[1] = x[1] + b1.T
    return out


_PROGRAMS = {}


def _get_program(S):
    if S not in _PROGRAMS:
        _PROGRAMS[S] = build_program(S)
    return _PROGRAMS[S]


def run(x, w_in, w_out, rms_w, trace=False):
    from concourse.bass_utils import run_bass_kernel_spmd
    nc = _get_program(x.shape[1])
    in_maps = host_prepare(x, w_in, w_out, rms_w)
    res = run_bass_kernel_spmd(nc, in_maps, list(range(NCORES)), trace=trace)
    return assemble(x, res.results), res


def kernel(x, w_in, w_out, rms_w):
    out, _ = run(np.asarray(x), np.asarray(w_in), np.asarray(w_out),
                 np.asarray(rms_w))
    return out



# revision 3
# speedup vs baseline: 2.0016x; 2.0016x over previous
"""GroupedQueryAttention Trainium2 kernel (8-core SPMD), v2.

Reference op: RMSNorm -> in-proj (q/k/v) -> RoPE -> causal GQA attention
-> out-proj -> residual.  b=2, s=2048, d_model=2048, 32 q-heads / 8 KV
groups, head dim 64, fp32.

Sharding: core c handles batch b = c//4 and KV groups (2j, 2j+1), j = c%4.
Each core computes the in-projection restricted to its 8 heads' channels,
attention for its 8 heads, and a partial out-projection (row-parallel).
The host sums the 4 partials per batch and adds the residual.

v2 changes vs v1 (1.05 ms):
  * No DRAM bounces: 1/sqrt and 1/x computed as Exp(-0.5*Ln(x)) /
    Exp(-Ln(x)) on ACT (same table set as the softmax Exp -> zero
    ACT_TABLE_LOAD switches); partition broadcasts via gpsimd ucode.
  * w_out resident in SBUF (was: 256 re-loads of 32KB tiles).
  * Attention for chunk c emitted inside chunk c (no chunk lag) ->
    much smaller serial tail.
  * qk PSUM double-buffered so QK(t+1) overlaps exp(t).
  * f16 rope tables / ops (2x DVE), f16 yT output (half DMA + 2x cast).
  * V transposed directly into vA/vB via two [64,128] DMA transposes.
  * x / w_in / w_out / cos / sin loaded with one large DMA each.
"""

import numpy as np
from contextlib import ExitStack

import concourse.bass as bass
from concourse import bacc as _bacc
import concourse.mybir as mybir
import concourse.tile as tile
from concourse.bass import ts

f32 = mybir.dt.float32
f16 = mybir.dt.float16
MDT = f16
MDT_NP = np.float16
AF = mybir.ActivationFunctionType
ALU = mybir.AluOpType

D = 2048          # model dim
CH = 768          # per-core in-proj channels (8 q heads + 2 k + 2 v)
TOKC = 512        # token chunk
NKT = D // 128    # 16 k-tiles over model dim
RMS_EPS = 1e-6
ROPE_THETA = 10000.0
NCORES = 8


def build_program(S=2048):
    NCH = S // TOKC          # token chunks
    NSK = S // 128           # sk tiles
    nc = _bacc.Bacc(None)

    xT_d = nc.dram_tensor("xT", [D, S], MDT, kind="ExternalInput")
    w_inT_d = nc.dram_tensor("w_inT", [D, CH], MDT, kind="ExternalInput")
    w_outT_d = nc.dram_tensor("w_outT", [512, D], MDT, kind="ExternalInput")
    cos_d = nc.dram_tensor("cos_t", [128, S], MDT, kind="ExternalInput")
    sin_d = nc.dram_tensor("sin_t", [128, S], MDT, kind="ExternalInput")
    tri_d = nc.dram_tensor("tri", [128, 128], MDT, kind="ExternalInput")
    oner_d = nc.dram_tensor("oner", [1], MDT, kind="ExternalInput")
    eps_d = nc.dram_tensor("epsc", [1], f32, kind="ExternalInput")
    yT_d = nc.dram_tensor("yT", [D, S], MDT, kind="ExternalOutput")

    with tile.TileContext(nc) as tc, ExitStack() as ctx:
        sb = ctx.enter_context(tc.tile_pool(name="sb", bufs=1))
        sbs = ctx.enter_context(tc.tile_pool(name="sbs", bufs=2))

        # persistent SBUF
        w_in_sb = sb.tile([128, NKT, CH], MDT, name="w_in_sb")
        w_out_sb = sb.tile([128, 4, D], MDT, name="w_out_sb")
        qkv = sb.tile([128, 6, S], MDT, name="qkv")    # 0-3 q pairs, 4 k, 5 v
        oT = sb.tile([128, 4, S], MDT, name="oT")
        vA = sb.tile([128, NSK, 65], MDT, name="vA")   # V^T + ones col, group 0
        vB = sb.tile([128, NSK, 65], MDT, name="vB")   # group 1
        cos_sb = sb.tile([128, S], MDT, name="cos_sb")
        sin_sb = sb.tile([128, S], MDT, name="sin_sb")
        tri_sb = sb.tile([128, 128], MDT, name="tri_sb")
        ones_sb = sb.tile([128, 1], MDT, name="ones_sb")
        eps_sb = sb.tile([1, 1], f32, name="eps_sb")

        nc.sync.dma_start(w_in_sb[:], w_inT_d.rearrange("(o p) c -> p o c", p=128))
        nc.sync.dma_start(w_out_sb[:], w_outT_d.rearrange("(o p) m -> p o m", p=128))
        nc.sync.dma_start(cos_sb[:], cos_d[:])
        nc.sync.dma_start(sin_sb[:], sin_d[:])
        nc.sync.dma_start(tri_sb[:], tri_d[:])
        nc.sync.dma_start(ones_sb[:], oner_d[None, :].to_broadcast((128, 1)))
        nc.sync.dma_start(vA[:, :, 64:65], oner_d[None, None, :].to_broadcast((128, NSK, 1)))
        nc.sync.dma_start(vB[:, :, 64:65], oner_d[None, None, :].to_broadcast((128, NSK, 1)))
        nc.sync.dma_start(eps_sb[:], eps_d[None, :])

        # PSUM: mm(2) + qk(2x2) + av(2) = 8 banks
        with tc.tile_pool(name="ps", bufs=1, space="PSUM") as ps:

            def emit_prelude(c):
                cs = slice(c * TOKC, (c + 1) * TOKC)
                xt = sbs.tile([128, NKT, TOKC], MDT, tag="xt", bufs=2,
                              name=f"xt_{c}")
                nc.sync.dma_start(
                    xt[:], xT_d.rearrange("(o p) (n t) -> p o n t", p=128,
                                          t=TOKC)[:, :, c, :])
                # sum of squares -> inv_rms = exp(-0.5*ln(ss/D + eps))
                ss = ps.tile([1, TOKC], f32, tag="mm", bufs=2, name=f"ss_{c}")
                for kt in range(NKT):
                    xsq = sbs.tile([128, TOKC], MDT, tag="xsq", bufs=2,
                                   name=f"xsq_{c}_{kt}")
                    nc.vector.tensor_tensor(xsq[:], xt[:, kt, :], xt[:, kt, :],
                                            ALU.mult)
                    nc.tensor.matmul(ss[:], ones_sb[:], xsq[:],
                                     start=(kt == 0), stop=(kt == NKT - 1))
                lnms = sbs.tile([1, TOKC], f32, tag="lnms", bufs=2,
                                name=f"lnms_{c}")
                nc.scalar.activation(lnms[:], ss[:], AF.Ln,
                                     bias=eps_sb[:], scale=1.0 / D)
                inv_row = sbs.tile([1, TOKC], MDT, tag="invr", bufs=2,
                                   name=f"invr_{c}")
                nc.scalar.activation(inv_row[:], lnms[:], AF.Exp, scale=-0.5)
                inv128 = sbs.tile([128, TOKC], MDT, tag="inv128", bufs=2,
                                  name=f"inv128_{c}")
                nc.gpsimd.partition_broadcast(inv128[:], inv_row[:], channels=128)
                cosi = sbs.tile([128, TOKC], MDT, tag="cosi", bufs=2,
                                name=f"cosi_{c}")
                nc.vector.tensor_tensor(cosi[:], cos_sb[:, cs], inv128[:],
                                        ALU.mult)
                sini = sbs.tile([128, TOKC], MDT, tag="sini", bufs=2,
                                name=f"sini_{c}")
                nc.vector.tensor_tensor(sini[:], sin_sb[:, cs], inv128[:],
                                        ALU.mult)
                return xt, cosi, sini, inv128

            def emit_inproj_m(c, m, state):
                cs = slice(c * TOKC, (c + 1) * TOKC)
                xt, cosi, sini, inv128 = state
                ip = ps.tile([128, TOKC], f32, tag="mm", bufs=2,
                             name=f"ip{m}_{c}")
                for kt in range(NKT):
                    nc.tensor.matmul(ip[:], w_in_sb[:, kt, ts(m, 128)],
                                     xt[:, kt, :],
                                     start=(kt == 0), stop=(kt == NKT - 1))
                nc.vector.tensor_copy(qkv[:, m, cs], ip[:])
                if m < 5:
                    # rope in place, inv_rms folded into the tables.
                    tmp = sbs.tile([128, TOKC], MDT, tag="rtmp", bufs=2,
                                   name=f"rtmp_{c}_{m}")
                    for dst, src in ((0, 32), (32, 0), (64, 96), (96, 64)):
                        nc.vector.tensor_tensor(
                            tmp[dst:dst + 32, :],
                            qkv[src:src + 32, m, cs],
                            sini[src:src + 32, :],
                            ALU.mult,
                        )
                    nc.vector.tensor_tensor(qkv[:, m, cs], qkv[:, m, cs],
                                            cosi[:], ALU.mult)
                    nc.vector.tensor_tensor(qkv[:, m, cs], qkv[:, m, cs],
                                            tmp[:], ALU.add)
                else:
                    # V: scale by inv_rms, then transpose into vA/vB
                    nc.vector.tensor_tensor(qkv[:, 5, cs], qkv[:, 5, cs],
                                            inv128[:], ALU.mult)
                    for tl in range(TOKC // 128):
                        t = c * (TOKC // 128) + tl
                        vtt = sbs.tile([128, 128], MDT, tag="vtt", bufs=2,
                                       name=f"vtt_{t}")
                        nc.sync.dma_start(vtt[:], qkv[:, 5, ts(t, 128)],
                                          transpose=True)
                        nc.vector.tensor_copy(vA[:, t, 0:64], vtt[:, 0:64])
                        nc.vector.tensor_copy(vB[:, t, 0:64], vtt[:, 64:128])

            def emit_attn_pair(c, p):
                cs = slice(c * TOKC, (c + 1) * TOKC)
                n_t = 4 * (c + 1)
                avA = ps.tile([65, TOKC], f32, tag="av", bufs=2,
                              name=f"avA_{c}_{p}")
                avB = ps.tile([65, TOKC], f32, tag="av", bufs=2,
                              name=f"avB_{c}_{p}")
                for t in range(n_t):
                    j0 = max(0, t - 4 * c) * 128
                    qk = ps.tile([128, 2, TOKC], f32, tag="qk", bufs=2,
                                 name=f"qk_{c}_{p}_{t}")
                    nc.tensor.matmul(
                        qk[:, 0, j0:],
                        qkv[0:64, 4, ts(t, 128)],
                        qkv[0:64, p, c * TOKC + j0:(c + 1) * TOKC],
                        start=True, stop=True,
                    )
                    nc.tensor.matmul(
                        qk[:, 1, j0:],
                        qkv[64:128, 4, ts(t, 128)],
                        qkv[64:128, p, c * TOKC + j0:(c + 1) * TOKC],
                        start=True, stop=True,
                    )
                    e = sbs.tile([128, 2, TOKC], MDT, tag="e", bufs=6,
                                 name=f"e_{c}_{p}_{t}")
                    nc.scalar.activation(e[:, :, j0:], qk[:, :, j0:], AF.Exp)
                    if t >= 4 * c:  # diagonal tile: causal mask
                        for h in (0, 1):
                            nc.vector.tensor_tensor(
                                e[:, h, j0:j0 + 128],
                                e[:, h, j0:j0 + 128],
                                tri_sb[:],
                                ALU.mult,
                            )
                    nc.tensor.matmul(avA[:, j0:], vA[:, t, :], e[:, 0, j0:],
                                     start=(t == 0), stop=(t == n_t - 1))
                    nc.tensor.matmul(avB[:, j0:], vB[:, t, :], e[:, 1, j0:],
                                     start=(t == 0), stop=(t == n_t - 1))
                # softmax denominators: row 64 of each AV psum.
                # 1/d = exp(-ln(d)) on ACT (same table set as Exp).
                lnd = sbs.tile([1, 2, TOKC], f32, tag="lnd", bufs=2,
                               name=f"lnd_{c}_{p}")
                nc.scalar.activation(lnd[:, 0, :], avA[64:65, :], AF.Ln)
                nc.scalar.activation(lnd[:, 1, :], avB[64:65, :], AF.Ln)
                invd = sbs.tile([1, 2, TOKC], f32, tag="invd", bufs=2,
                                name=f"invd_{c}_{p}")
                nc.scalar.activation(invd[:], lnd[:], AF.Exp, scale=-1.0)
                dbA = sbs.tile([64, TOKC], f32, tag="dbA", bufs=2,
                               name=f"dbA_{c}_{p}")
                nc.gpsimd.partition_broadcast(dbA[:], invd[:, 0, :], channels=64)
                dbB = sbs.tile([64, TOKC], f32, tag="dbB", bufs=2,
                               name=f"dbB_{c}_{p}")
                nc.gpsimd.partition_broadcast(dbB[:], invd[:, 1, :], channels=64)
                nc.vector.tensor_tensor(oT[0:64, p, cs], avA[0:64, :],
                                        dbA[:], ALU.mult)
                nc.vector.tensor_tensor(oT[64:128, p, cs], avB[0:64, :],
                                        dbB[:], ALU.mult)

            def emit_outproj(c):
                cs = slice(c * TOKC, (c + 1) * TOKC)
                for m in range(16):
                    op = ps.tile([128, TOKC], f32, tag="mm", bufs=2,
                                 name=f"op_{c}_{m}")
                    for kt in range(4):
                        nc.tensor.matmul(op[:], w_out_sb[:, kt, ts(m, 128)],
                                         oT[:, kt, cs],
                                         start=(kt == 0), stop=(kt == 3))
                    yt = sbs.tile([128, TOKC], MDT, tag="yt", bufs=3,
                                  name=f"yt_{c}_{m}")
                    nc.vector.tensor_copy(yt[:], op[:])
                    nc.gpsimd.dma_start(yT_d[ts(m, 128), cs], yt[:])

            for c in range(NCH):
                st = emit_prelude(c)
                emit_inproj_m(c, 4, st)
                emit_inproj_m(c, 5, st)
                for p in range(4):
                    emit_inproj_m(c, p, st)
                    emit_attn_pair(c, p)
                emit_outproj(c)

    nc.finalize()
    return nc


# ------------------------------- host side ----------------------------------

def _rope_tables(S):
    inv_freq = ROPE_THETA ** (-np.arange(0, 64, 2, dtype=np.float64) / 64.0)
    ang = np.arange(S, dtype=np.float64)[:, None] * inv_freq[None, :]  # [S, 32]
    cosb = np.cos(ang).T.astype(np.float32)   # [32, S]
    sinb = np.sin(ang).T.astype(np.float32)
    cos128 = np.tile(cosb, (4, 1))                               # [128, S]
    sin128 = np.concatenate([sinb, -sinb, sinb, -sinb], axis=0)  # [128, S]
    return np.ascontiguousarray(cos128), np.ascontiguousarray(sin128)


def host_prepare(x, w_in, w_out, rms_w):
    """Build the 8 per-core input maps."""
    S = x.shape[1]
    x = np.asarray(x, dtype=np.float32)
    w_eff = np.asarray(w_in, dtype=np.float32) * np.asarray(rms_w, np.float32)[None, :]
    w_out = np.asarray(w_out, dtype=np.float32)
    cos128, sin128 = _rope_tables(S)
    tri = np.ascontiguousarray(np.triu(np.ones((128, 128), dtype=np.float32)))
    qscale = np.float32(64 ** -0.5)

    in_maps = []
    for core in range(NCORES):
        b, j = divmod(core, 4)
        g0, g1 = 2 * j, 2 * j + 1
        rows = []
        for p in range(4):
            for g in (g0, g1):
                rows.extend(range((g * 4 + p) * 64, (g * 4 + p) * 64 + 64))
        for g in (g0, g1):
            rows.extend(range(2048 + g * 64, 2048 + g * 64 + 64))
        for g in (g0, g1):
            rows.extend(range(2560 + g * 64, 2560 + g * 64 + 64))
        w_slice = w_eff[rows, :].copy()
        w_slice[:512, :] *= qscale
        cols = []
        for p in range(4):
            for g in (g0, g1):
                cols.extend(range((g * 4 + p) * 64, (g * 4 + p) * 64 + 64))
        in_maps.append({
            "xT": np.ascontiguousarray(x[b].T).astype(MDT_NP),
            "w_inT": np.ascontiguousarray(w_slice.T).astype(MDT_NP),
            "w_outT": np.ascontiguousarray(w_out[:, cols].T).astype(MDT_NP),
            "cos_t": cos128.astype(MDT_NP),
            "sin_t": sin128.astype(MDT_NP),
            "tri": tri.astype(MDT_NP),
            "oner": np.ones(1, dtype=MDT_NP),
            "epsc": np.full(1, RMS_EPS, dtype=np.float32),
        })
    return in_maps


def assemble(x, results):
    x = np.asarray(x, dtype=np.float32)
    b0 = sum(np.asarray(results[i]["yT"], dtype=np.float32) for i in range(4))
    b1 = sum(np.asarray(results[i]["yT"], dtype=np.float32) for i in range(4, 8))
    out = np.empty_like(x)
    out[0] = x[0] + b0.T
    out[1] = x[1] + b1.T
    return out


_PROGRAMS = {}


def _get_program(S):
    if S not in _PROGRAMS:
        _PROGRAMS[S] = build_program(S)
    return _PROGRAMS[S]


def run(x, w_in, w_out, rms_w, trace=False):
    from concourse.bass_utils import run_bass_kernel_spmd
    nc = _get_program(x.shape[1])
    in_maps = host_prepare(x, w_in, w_out, rms_w)
    res = run_bass_kernel_spmd(nc, in_maps, list(range(NCORES)), trace=trace)
    return assemble(x, res.results), res


def kernel(x, w_in, w_out, rms_w):
    out, _ = run(np.asarray(x), np.asarray(w_in), np.asarray(w_out),
                 np.asarray(rms_w))
    return out


# revision 8
# speedup vs baseline: 2.0472x; 1.0228x over previous
"""GroupedQueryAttention Trainium2 kernel (8-core SPMD), v2.

Reference op: RMSNorm -> in-proj (q/k/v) -> RoPE -> causal GQA attention
-> out-proj -> residual.  b=2, s=2048, d_model=2048, 32 q-heads / 8 KV
groups, head dim 64, fp32.

Sharding: core c handles batch b = c//4 and KV groups (2j, 2j+1), j = c%4.
Each core computes the in-projection restricted to its 8 heads' channels,
attention for its 8 heads, and a partial out-projection (row-parallel).
The host sums the 4 partials per batch and adds the residual.

v2 changes vs v1 (1.05 ms):
  * No DRAM bounces: 1/sqrt and 1/x computed as Exp(-0.5*Ln(x)) /
    Exp(-Ln(x)) on ACT (same table set as the softmax Exp -> zero
    ACT_TABLE_LOAD switches); partition broadcasts via gpsimd ucode.
  * w_out resident in SBUF (was: 256 re-loads of 32KB tiles).
  * Attention for chunk c emitted inside chunk c (no chunk lag) ->
    much smaller serial tail.
  * qk PSUM double-buffered so QK(t+1) overlaps exp(t).
  * f16 rope tables / ops (2x DVE), f16 yT output (half DMA + 2x cast).
  * V transposed directly into vA/vB via two [64,128] DMA transposes.
  * x / w_in / w_out / cos / sin loaded with one large DMA each.
"""

import numpy as np
from contextlib import ExitStack

import concourse.bass as bass
from concourse import bacc as _bacc
import concourse.mybir as mybir
import concourse.tile as tile
from concourse.bass import ts

f32 = mybir.dt.float32
f16 = mybir.dt.float16
MDT = f16
MDT_NP = np.float16
AF = mybir.ActivationFunctionType
ALU = mybir.AluOpType

D = 2048          # model dim
CH = 768          # per-core in-proj channels (8 q heads + 2 k + 2 v)
TOKC = 512        # token chunk
NKT = D // 128    # 16 k-tiles over model dim
RMS_EPS = 1e-6
ROPE_THETA = 10000.0
NCORES = 8


def build_program(S=2048):
    NCH = S // TOKC          # token chunks
    NSK = S // 128           # sk tiles
    nc = _bacc.Bacc(None)

    xT_d = nc.dram_tensor("xT", [D, S], MDT, kind="ExternalInput")
    w_inT_d = nc.dram_tensor("w_inT", [D, CH], MDT, kind="ExternalInput")
    w_outT_d = nc.dram_tensor("w_outT", [512, D], MDT, kind="ExternalInput")
    cos_d = nc.dram_tensor("cos_t", [128, S], MDT, kind="ExternalInput")
    sin_d = nc.dram_tensor("sin_t", [128, S], MDT, kind="ExternalInput")
    tri_d = nc.dram_tensor("tri", [128, 128], MDT, kind="ExternalInput")
    oner_d = nc.dram_tensor("oner", [1], MDT, kind="ExternalInput")
    eps_d = nc.dram_tensor("epsc", [1], f32, kind="ExternalInput")
    yT_d = nc.dram_tensor("yT", [D, S], MDT, kind="ExternalOutput")

    with tile.TileContext(nc) as tc, ExitStack() as ctx:
        sb = ctx.enter_context(tc.tile_pool(name="sb", bufs=1))
        sbs = ctx.enter_context(tc.tile_pool(name="sbs", bufs=2))

        # persistent SBUF
        w_in_sb = sb.tile([128, NKT, CH], MDT, name="w_in_sb")
        w_out_sb = sb.tile([128, 4, D], MDT, name="w_out_sb")
        qkv = sb.tile([128, 6, S], MDT, name="qkv")    # 0-3 q pairs, 4 k, 5 v
        oT = sb.tile([128, 4, S], MDT, name="oT")
        vA = sb.tile([128, NSK, 65], MDT, name="vA")   # V^T + ones col, group 0
        vB = sb.tile([128, NSK, 65], MDT, name="vB")   # group 1
        cos_sb = sb.tile([128, S], MDT, name="cos_sb")
        sin_sb = sb.tile([128, S], MDT, name="sin_sb")
        tri_sb = sb.tile([128, 128], MDT, name="tri_sb")
        ones_sb = sb.tile([128, 1], MDT, name="ones_sb")
        eps_sb = sb.tile([1, 1], f32, name="eps_sb")

        # Pin the ACT table set to natural_log_exp_and_others (id 6): it
        # covers both Ln and Exp, so walrus never re-loads tables mid-kernel.
        nc.scalar.add_instruction(mybir.InstLoadActFuncSet(
            name=nc.get_next_instruction_name(), act_func_set_id=6,
            ins=[], outs=[]))
        nc.sync.dma_start(w_in_sb[:], w_inT_d.rearrange("(o p) c -> p o c", p=128))
        nc.sync.dma_start(w_out_sb[:], w_outT_d.rearrange("(o p) m -> p o m", p=128))
        nc.sync.dma_start(cos_sb[:], cos_d[:])
        nc.sync.dma_start(sin_sb[:], sin_d[:])
        nc.sync.dma_start(tri_sb[:], tri_d[:])
        nc.sync.dma_start(ones_sb[:], oner_d[None, :].to_broadcast((128, 1)))
        nc.sync.dma_start(vA[:, :, 64:65], oner_d[None, None, :].to_broadcast((128, NSK, 1)))
        nc.sync.dma_start(vB[:, :, 64:65], oner_d[None, None, :].to_broadcast((128, NSK, 1)))
        nc.sync.dma_start(eps_sb[:], eps_d[None, :])

        # PSUM: mm(2) + qk(2x2) + av(2) = 8 banks
        with tc.tile_pool(name="ps", bufs=1, space="PSUM") as ps:

            def emit_prelude(c):
                cs = slice(c * TOKC, (c + 1) * TOKC)
                xt = sbs.tile([128, NKT, TOKC], MDT, tag="xt", bufs=3,
                              name=f"xt_{c}")
                xT_v = xT_d.rearrange("(o p) (n t) -> p o n t", p=128, t=TOKC)
                nc.sync.dma_start(xt[:, 0:8, :], xT_v[:, 0:8, c, :])
                nc.sync.dma_start(xt[:, 8:16, :], xT_v[:, 8:16, c, :])
                # sum of squares -> inv_rms = exp(-0.5*ln(ss/D + eps))
                ss = ps.tile([1, TOKC], f32, tag="mm", bufs=2, name=f"ss_{c}")
                for kt in range(NKT):
                    xsq = sbs.tile([128, TOKC], MDT, tag="xsq", bufs=2,
                                   name=f"xsq_{c}_{kt}")
                    nc.vector.tensor_tensor(xsq[:], xt[:, kt, :], xt[:, kt, :],
                                            ALU.mult)
                    nc.tensor.matmul(ss[:], ones_sb[:], xsq[:],
                                     start=(kt == 0), stop=(kt == NKT - 1))
                lnms = sbs.tile([1, TOKC], f32, tag="lnms", bufs=2,
                                name=f"lnms_{c}")
                nc.scalar.activation(lnms[:], ss[:], AF.Ln,
                                     bias=eps_sb[:], scale=1.0 / D)
                inv_row = sbs.tile([1, TOKC], MDT, tag="invr", bufs=2,
                                   name=f"invr_{c}")
                nc.scalar.activation(inv_row[:], lnms[:], AF.Exp, scale=-0.5)
                inv128 = sbs.tile([128, TOKC], MDT, tag="inv128", bufs=2,
                                  name=f"inv128_{c}")
                nc.gpsimd.partition_broadcast(inv128[:], inv_row[:], channels=128)
                cosi = sbs.tile([128, TOKC], MDT, tag="cosi", bufs=2,
                                name=f"cosi_{c}")
                nc.vector.tensor_tensor(cosi[:], cos_sb[:, cs], inv128[:],
                                        ALU.mult)
                sini = sbs.tile([128, TOKC], MDT, tag="sini", bufs=2,
                                name=f"sini_{c}")
                nc.vector.tensor_tensor(sini[:], sin_sb[:, cs], inv128[:],
                                        ALU.mult)
                return xt, cosi, sini, inv128

            def emit_inproj_m(c, m, state):
                cs = slice(c * TOKC, (c + 1) * TOKC)
                xt, cosi, sini, inv128 = state
                ip = ps.tile([128, TOKC], f32, tag="mm", bufs=2,
                             name=f"ip{m}_{c}")
                for kt in range(NKT):
                    nc.tensor.matmul(ip[:], w_in_sb[:, kt, ts(m, 128)],
                                     xt[:, kt, :],
                                     start=(kt == 0), stop=(kt == NKT - 1))
                nc.vector.tensor_copy(qkv[:, m, cs], ip[:])
                if m < 5:
                    # rope in place, inv_rms folded into the tables.
                    tmp = sbs.tile([128, TOKC], MDT, tag="rtmp", bufs=2,
                                   name=f"rtmp_{c}_{m}")
                    for dst, src in ((0, 32), (32, 0), (64, 96), (96, 64)):
                        nc.vector.tensor_tensor(
                            tmp[dst:dst + 32, :],
                            qkv[src:src + 32, m, cs],
                            sini[src:src + 32, :],
                            ALU.mult,
                        )
                    nc.vector.tensor_tensor(qkv[:, m, cs], qkv[:, m, cs],
                                            cosi[:], ALU.mult)
                    nc.vector.tensor_tensor(qkv[:, m, cs], qkv[:, m, cs],
                                            tmp[:], ALU.add)
                else:
                    # V: scale by inv_rms, then transpose into vA/vB
                    nc.vector.tensor_tensor(qkv[:, 5, cs], qkv[:, 5, cs],
                                            inv128[:], ALU.mult)
                    for tl in range(TOKC // 128):
                        t = c * (TOKC // 128) + tl
                        vtt = sbs.tile([128, 128], MDT, tag="vtt", bufs=2,
                                       name=f"vtt_{t}")
                        nc.sync.dma_start(vtt[:], qkv[:, 5, ts(t, 128)],
                                          transpose=True)
                        nc.vector.tensor_copy(vA[:, t, 0:64], vtt[:, 0:64])
                        nc.vector.tensor_copy(vB[:, t, 0:64], vtt[:, 64:128])

            def emit_attn_pair(c, p):
                cs = slice(c * TOKC, (c + 1) * TOKC)
                n_t = 4 * (c + 1)
                avA = ps.tile([65, TOKC], f32, tag="av", bufs=2,
                              name=f"avA_{c}_{p}")
                avB = ps.tile([65, TOKC], f32, tag="av", bufs=2,
                              name=f"avB_{c}_{p}")
                for t in range(n_t):
                    j0 = max(0, t - 4 * c) * 128
                    qk = ps.tile([128, 2, TOKC], f32, tag="qk", bufs=2,
                                 name=f"qk_{c}_{p}_{t}")
                    nc.tensor.matmul(
                        qk[:, 0, j0:],
                        qkv[0:64, 4, ts(t, 128)],
                        qkv[0:64, p, c * TOKC + j0:(c + 1) * TOKC],
                        start=True, stop=True,
                    )
                    nc.tensor.matmul(
                        qk[:, 1, j0:],
                        qkv[64:128, 4, ts(t, 128)],
                        qkv[64:128, p, c * TOKC + j0:(c + 1) * TOKC],
                        start=True, stop=True,
                    )
                    e = sbs.tile([128, 2, TOKC], MDT, tag="e", bufs=4,
                                 name=f"e_{c}_{p}_{t}")
                    nc.scalar.activation(e[:, :, j0:], qk[:, :, j0:], AF.Exp)
                    if t >= 4 * c:  # diagonal tile: causal mask
                        for h in (0, 1):
                            nc.vector.tensor_tensor(
                                e[:, h, j0:j0 + 128],
                                e[:, h, j0:j0 + 128],
                                tri_sb[:],
                                ALU.mult,
                            )
                    nc.tensor.matmul(avA[:, j0:], vA[:, t, :], e[:, 0, j0:],
                                     start=(t == 0), stop=(t == n_t - 1))
                    nc.tensor.matmul(avB[:, j0:], vB[:, t, :], e[:, 1, j0:],
                                     start=(t == 0), stop=(t == n_t - 1))
                # Evacuate AV PSUM to SBUF immediately so the next pair's AV
                # accumulation can start while the softmax denominator chain
                # (Ln/Exp/broadcast) runs against the SBUF copy.
                avSA = sbs.tile([65, TOKC], f32, tag="avS", bufs=4,
                                name=f"avSA_{c}_{p}")
                nc.vector.tensor_copy(avSA[:], avA[:])
                avSB = sbs.tile([65, TOKC], f32, tag="avS", bufs=4,
                                name=f"avSB_{c}_{p}")
                nc.vector.tensor_copy(avSB[:], avB[:])
                # softmax denominators: row 64. 1/d = exp(-ln(d)) on ACT
                # (same table set as Exp -> no table reload).
                lnd = sbs.tile([1, 2, TOKC], f32, tag="lnd", bufs=2,
                               name=f"lnd_{c}_{p}")
                nc.scalar.activation(lnd[:, 0, :], avSA[64:65, :], AF.Ln)
                nc.scalar.activation(lnd[:, 1, :], avSB[64:65, :], AF.Ln)
                invd = sbs.tile([1, 2, TOKC], f32, tag="invd", bufs=2,
                                name=f"invd_{c}_{p}")
                nc.scalar.activation(invd[:], lnd[:], AF.Exp, scale=-1.0)
                dbA = sbs.tile([64, TOKC], f32, tag="dbA", bufs=2,
                               name=f"dbA_{c}_{p}")
                nc.gpsimd.partition_broadcast(dbA[:], invd[:, 0, :], channels=64)
                dbB = sbs.tile([64, TOKC], f32, tag="dbB", bufs=2,
                               name=f"dbB_{c}_{p}")
                nc.gpsimd.partition_broadcast(dbB[:], invd[:, 1, :], channels=64)
                nc.vector.tensor_tensor(oT[0:64, p, cs], avSA[0:64, :],
                                        dbA[:], ALU.mult)
                nc.vector.tensor_tensor(oT[64:128, p, cs], avSB[0:64, :],
                                        dbB[:], ALU.mult)

            def emit_outproj(c):
                cs = slice(c * TOKC, (c + 1) * TOKC)
                for m in range(16):
                    op = ps.tile([128, TOKC], f32, tag="mm", bufs=2,
                                 name=f"op_{c}_{m}")
                    for kt in range(4):
                        nc.tensor.matmul(op[:], w_out_sb[:, kt, ts(m, 128)],
                                         oT[:, kt, cs],
                                         start=(kt == 0), stop=(kt == 3))
                    yt = sbs.tile([128, TOKC], MDT, tag="yt", bufs=2,
                                  name=f"yt_{c}_{m}")
                    nc.vector.tensor_copy(yt[:], op[:])
                    nc.gpsimd.dma_start(yT_d[ts(m, 128), cs], yt[:])

            for c in range(NCH):
                st = emit_prelude(c)
                emit_inproj_m(c, 4, st)
                emit_inproj_m(c, 5, st)
                for p in range(4):
                    emit_inproj_m(c, p, st)
                    emit_attn_pair(c, p)
                emit_outproj(c)

    nc.finalize()
    return nc


# ------------------------------- host side ----------------------------------

def _rope_tables(S):
    inv_freq = ROPE_THETA ** (-np.arange(0, 64, 2, dtype=np.float64) / 64.0)
    ang = np.arange(S, dtype=np.float64)[:, None] * inv_freq[None, :]  # [S, 32]
    cosb = np.cos(ang).T.astype(np.float32)   # [32, S]
    sinb = np.sin(ang).T.astype(np.float32)
    cos128 = np.tile(cosb, (4, 1))                               # [128, S]
    sin128 = np.concatenate([sinb, -sinb, sinb, -sinb], axis=0)  # [128, S]
    return np.ascontiguousarray(cos128), np.ascontiguousarray(sin128)


def host_prepare(x, w_in, w_out, rms_w):
    """Build the 8 per-core input maps."""
    S = x.shape[1]
    x = np.asarray(x, dtype=np.float32)
    w_eff = np.asarray(w_in, dtype=np.float32) * np.asarray(rms_w, np.float32)[None, :]
    w_out = np.asarray(w_out, dtype=np.float32)
    cos128, sin128 = _rope_tables(S)
    tri = np.ascontiguousarray(np.triu(np.ones((128, 128), dtype=np.float32)))
    qscale = np.float32(64 ** -0.5)

    in_maps = []
    for core in range(NCORES):
        b, j = divmod(core, 4)
        g0, g1 = 2 * j, 2 * j + 1
        rows = []
        for p in range(4):
            for g in (g0, g1):
                rows.extend(range((g * 4 + p) * 64, (g * 4 + p) * 64 + 64))
        for g in (g0, g1):
            rows.extend(range(2048 + g * 64, 2048 + g * 64 + 64))
        for g in (g0, g1):
            rows.extend(range(2560 + g * 64, 2560 + g * 64 + 64))
        w_slice = w_eff[rows, :].copy()
        w_slice[:512, :] *= qscale
        cols = []
        for p in range(4):
            for g in (g0, g1):
                cols.extend(range((g * 4 + p) * 64, (g * 4 + p) * 64 + 64))
        in_maps.append({
            "xT": np.ascontiguousarray(x[b].T).astype(MDT_NP),
            "w_inT": np.ascontiguousarray(w_slice.T).astype(MDT_NP),
            "w_outT": np.ascontiguousarray(w_out[:, cols].T).astype(MDT_NP),
            "cos_t": cos128.astype(MDT_NP),
            "sin_t": sin128.astype(MDT_NP),
            "tri": tri.astype(MDT_NP),
            "oner": np.ones(1, dtype=MDT_NP),
            "epsc": np.full(1, RMS_EPS, dtype=np.float32),
        })
    return in_maps


def assemble(x, results):
    x = np.asarray(x, dtype=np.float32)
    b0 = sum(np.asarray(results[i]["yT"], dtype=np.float32) for i in range(4))
    b1 = sum(np.asarray(results[i]["yT"], dtype=np.float32) for i in range(4, 8))
    out = np.empty_like(x)
    out[0] = x[0] + b0.T
    out[1] = x[1] + b1.T
    return out


_PROGRAMS = {}


def _get_program(S):
    if S not in _PROGRAMS:
        _PROGRAMS[S] = build_program(S)
    return _PROGRAMS[S]


def run(x, w_in, w_out, rms_w, trace=False):
    from concourse.bass_utils import run_bass_kernel_spmd
    nc = _get_program(x.shape[1])
    in_maps = host_prepare(x, w_in, w_out, rms_w)
    res = run_bass_kernel_spmd(nc, in_maps, list(range(NCORES)), trace=trace)
    return assemble(x, res.results), res


def kernel(x, w_in, w_out, rms_w):
    out, _ = run(np.asarray(x), np.asarray(w_in), np.asarray(w_out),
                 np.asarray(rms_w))
    return out


# revision 11
# speedup vs baseline: 2.1459x; 1.0482x over previous
"""GroupedQueryAttention Trainium2 kernel (8-core SPMD), v2.

Reference op: RMSNorm -> in-proj (q/k/v) -> RoPE -> causal GQA attention
-> out-proj -> residual.  b=2, s=2048, d_model=2048, 32 q-heads / 8 KV
groups, head dim 64, fp32.

Sharding: core c handles batch b = c//4 and KV groups (2j, 2j+1), j = c%4.
Each core computes the in-projection restricted to its 8 heads' channels,
attention for its 8 heads, and a partial out-projection (row-parallel).
The host sums the 4 partials per batch and adds the residual.

v2 changes vs v1 (1.05 ms):
  * No DRAM bounces: 1/sqrt and 1/x computed as Exp(-0.5*Ln(x)) /
    Exp(-Ln(x)) on ACT (same table set as the softmax Exp -> zero
    ACT_TABLE_LOAD switches); partition broadcasts via gpsimd ucode.
  * w_out resident in SBUF (was: 256 re-loads of 32KB tiles).
  * Attention for chunk c emitted inside chunk c (no chunk lag) ->
    much smaller serial tail.
  * qk PSUM double-buffered so QK(t+1) overlaps exp(t).
  * f16 rope tables / ops (2x DVE), f16 yT output (half DMA + 2x cast).
  * V transposed directly into vA/vB via two [64,128] DMA transposes.
  * x / w_in / w_out / cos / sin loaded with one large DMA each.
"""

import numpy as np
from contextlib import ExitStack

import concourse.bass as bass
from concourse import bacc as _bacc
import concourse.mybir as mybir
import concourse.tile as tile
from concourse.bass import ts

f32 = mybir.dt.float32
f16 = mybir.dt.float16
MDT = f16
MDT_NP = np.float16
AF = mybir.ActivationFunctionType
ALU = mybir.AluOpType

D = 2048          # model dim
CH = 768          # per-core in-proj channels (8 q heads + 2 k + 2 v)
TOKC = 512        # token chunk
NKT = D // 128    # 16 k-tiles over model dim
RMS_EPS = 1e-6
ROPE_THETA = 10000.0
NCORES = 8


def build_program(S=2048):
    NCH = S // TOKC          # token chunks
    NSK = S // 128           # sk tiles
    nc = _bacc.Bacc(None)

    xT_d = nc.dram_tensor("xT", [D, S], MDT, kind="ExternalInput")
    w_inT_d = nc.dram_tensor("w_inT", [D, CH], MDT, kind="ExternalInput")
    w_outT_d = nc.dram_tensor("w_outT", [512, D], MDT, kind="ExternalInput")
    cos_d = nc.dram_tensor("cos_t", [128, S], MDT, kind="ExternalInput")
    sin_d = nc.dram_tensor("sin_t", [128, S], MDT, kind="ExternalInput")
    tri_d = nc.dram_tensor("tri", [128, 128], MDT, kind="ExternalInput")
    oner_d = nc.dram_tensor("oner", [1], MDT, kind="ExternalInput")
    eps_d = nc.dram_tensor("epsc", [1], f32, kind="ExternalInput")
    yT_d = nc.dram_tensor("yT", [D, S], MDT, kind="ExternalOutput")

    with tile.TileContext(nc) as tc, ExitStack() as ctx:
        sb = ctx.enter_context(tc.tile_pool(name="sb", bufs=1))
        sbs = ctx.enter_context(tc.tile_pool(name="sbs", bufs=2))

        # persistent SBUF
        w_in_sb = sb.tile([128, NKT, CH], MDT, name="w_in_sb")
        w_out_sb = sb.tile([128, 4, D], MDT, name="w_out_sb")
        qkv = sb.tile([128, 6, S], MDT, name="qkv")    # 0-3 q pairs, 4 k, 5 v
        oT = sb.tile([128, 4, S], MDT, name="oT")
        vA = sb.tile([128, NSK, 65], MDT, name="vA")   # V^T + ones col, group 0
        vB = sb.tile([128, NSK, 65], MDT, name="vB")   # group 1
        cos_sb = sb.tile([128, S], MDT, name="cos_sb")
        sin_sb = sb.tile([128, S], MDT, name="sin_sb")
        tri_sb = sb.tile([128, 128], MDT, name="tri_sb")
        ones_sb = sb.tile([128, 1], MDT, name="ones_sb")
        eps_sb = sb.tile([1, 1], f32, name="eps_sb")

        # Pin the ACT table set to natural_log_exp_and_others (id 6): it
        # covers both Ln and Exp, so walrus never re-loads tables mid-kernel.
        nc.scalar.add_instruction(mybir.InstLoadActFuncSet(
            name=nc.get_next_instruction_name(), act_func_set_id=6,
            ins=[], outs=[]))
        nc.sync.dma_start(w_in_sb[:], w_inT_d.rearrange("(o p) c -> p o c", p=128))
        nc.sync.dma_start(w_out_sb[:], w_outT_d.rearrange("(o p) m -> p o m", p=128))
        nc.sync.dma_start(cos_sb[:], cos_d[:])
        nc.sync.dma_start(sin_sb[:], sin_d[:])
        nc.sync.dma_start(tri_sb[:], tri_d[:])
        nc.sync.dma_start(ones_sb[:], oner_d[None, :].to_broadcast((128, 1)))
        nc.sync.dma_start(vA[:, :, 64:65], oner_d[None, None, :].to_broadcast((128, NSK, 1)))
        nc.sync.dma_start(vB[:, :, 64:65], oner_d[None, None, :].to_broadcast((128, NSK, 1)))
        nc.sync.dma_start(eps_sb[:], eps_d[None, :])

        # PSUM: mm(2) + qk(2x2) + av(2) = 8 banks
        with tc.tile_pool(name="ps", bufs=1, space="PSUM") as ps:

            xT_v = xT_d.rearrange("(o p) (n t) -> p o n t", p=128, t=TOKC)
            XT = {}
            ST = {}

            def load_x(c):
                xt = sbs.tile([128, NKT, TOKC], MDT, tag="xt", bufs=3,
                              name=f"xt_{c}")
                nc.sync.dma_start(xt[:, 0:8, :], xT_v[:, 0:8, c, :])
                nc.sync.dma_start(xt[:, 8:16, :], xT_v[:, 8:16, c, :])
                XT[c] = xt

            def emit_prelude(c):
                cs = slice(c * TOKC, (c + 1) * TOKC)
                xt = XT[c]
                # sum of squares -> inv_rms = exp(-0.5*ln(ss/D + eps))
                ss = ps.tile([1, TOKC], f32, tag="mm", bufs=2, name=f"ss_{c}")
                for kt in range(NKT):
                    xsq = sbs.tile([128, TOKC], MDT, tag="xsq", bufs=2,
                                   name=f"xsq_{c}_{kt}")
                    nc.vector.tensor_tensor(xsq[:], xt[:, kt, :], xt[:, kt, :],
                                            ALU.mult)
                    nc.tensor.matmul(ss[:], ones_sb[:], xsq[:],
                                     start=(kt == 0), stop=(kt == NKT - 1))
                lnms = sbs.tile([1, TOKC], f32, tag="lnms", bufs=2,
                                name=f"lnms_{c}")
                nc.scalar.activation(lnms[:], ss[:], AF.Ln,
                                     bias=eps_sb[:], scale=1.0 / D)
                inv_row = sbs.tile([1, TOKC], MDT, tag="invr", bufs=2,
                                   name=f"invr_{c}")
                nc.scalar.activation(inv_row[:], lnms[:], AF.Exp, scale=-0.5)
                inv128 = sbs.tile([128, TOKC], MDT, tag="inv128", bufs=2,
                                  name=f"inv128_{c}")
                nc.gpsimd.partition_broadcast(inv128[:], inv_row[:], channels=128)
                cosi = sbs.tile([128, TOKC], MDT, tag="cosi", bufs=2,
                                name=f"cosi_{c}")
                nc.vector.tensor_tensor(cosi[:], cos_sb[:, cs], inv128[:],
                                        ALU.mult)
                sini = sbs.tile([128, TOKC], MDT, tag="sini", bufs=2,
                                name=f"sini_{c}")
                nc.vector.tensor_tensor(sini[:], sin_sb[:, cs], inv128[:],
                                        ALU.mult)
                ST[c] = (cosi, sini, inv128)

            def emit_inproj_m(c, m):
                cs = slice(c * TOKC, (c + 1) * TOKC)
                xt = XT[c]
                cosi, sini, inv128 = ST[c]
                ip = ps.tile([128, TOKC], f32, tag="mm", bufs=2,
                             name=f"ip{m}_{c}")
                for kt in range(NKT):
                    nc.tensor.matmul(ip[:], w_in_sb[:, kt, ts(m, 128)],
                                     xt[:, kt, :],
                                     start=(kt == 0), stop=(kt == NKT - 1))
                nc.vector.tensor_copy(qkv[:, m, cs], ip[:])
                if m < 5:
                    # rope in place, inv_rms folded into the tables.
                    tmp = sbs.tile([128, TOKC], MDT, tag="rtmp", bufs=2,
                                   name=f"rtmp_{c}_{m}")
                    for dst, src in ((0, 32), (32, 0), (64, 96), (96, 64)):
                        nc.vector.tensor_tensor(
                            tmp[dst:dst + 32, :],
                            qkv[src:src + 32, m, cs],
                            sini[src:src + 32, :],
                            ALU.mult,
                        )
                    nc.vector.tensor_tensor(qkv[:, m, cs], qkv[:, m, cs],
                                            cosi[:], ALU.mult)
                    nc.vector.tensor_tensor(qkv[:, m, cs], qkv[:, m, cs],
                                            tmp[:], ALU.add)
                else:
                    # V: scale by inv_rms, then transpose into vA/vB
                    nc.vector.tensor_tensor(qkv[:, 5, cs], qkv[:, 5, cs],
                                            inv128[:], ALU.mult)
                    for tl in range(TOKC // 128):
                        t = c * (TOKC // 128) + tl
                        vtt = sbs.tile([128, 128], MDT, tag="vtt", bufs=2,
                                       name=f"vtt_{t}")
                        nc.sync.dma_start(vtt[:], qkv[:, 5, ts(t, 128)],
                                          transpose=True)
                        nc.vector.tensor_copy(vA[:, t, 0:64], vtt[:, 0:64])
                        nc.vector.tensor_copy(vB[:, t, 0:64], vtt[:, 64:128])

            def emit_attn_pair(c, p):
                cs = slice(c * TOKC, (c + 1) * TOKC)
                n_t = 4 * (c + 1)
                avA = ps.tile([65, TOKC], f32, tag="av", bufs=2,
                              name=f"avA_{c}_{p}")
                avB = ps.tile([65, TOKC], f32, tag="av", bufs=2,
                              name=f"avB_{c}_{p}")
                for t in range(n_t):
                    j0 = max(0, t - 4 * c) * 128
                    qk = ps.tile([128, 2, TOKC], f32, tag="qk", bufs=2,
                                 name=f"qk_{c}_{p}_{t}")
                    nc.tensor.matmul(
                        qk[:, 0, j0:],
                        qkv[0:64, 4, ts(t, 128)],
                        qkv[0:64, p, c * TOKC + j0:(c + 1) * TOKC],
                        start=True, stop=True,
                    )
                    nc.tensor.matmul(
                        qk[:, 1, j0:],
                        qkv[64:128, 4, ts(t, 128)],
                        qkv[64:128, p, c * TOKC + j0:(c + 1) * TOKC],
                        start=True, stop=True,
                    )
                    e = sbs.tile([128, 2, TOKC], MDT, tag="e", bufs=4,
                                 name=f"e_{c}_{p}_{t}")
                    nc.scalar.activation(e[:, :, j0:], qk[:, :, j0:], AF.Exp)
                    if t >= 4 * c:  # diagonal tile: causal mask
                        for h in (0, 1):
                            nc.vector.tensor_tensor(
                                e[:, h, j0:j0 + 128],
                                e[:, h, j0:j0 + 128],
                                tri_sb[:],
                                ALU.mult,
                            )
                    nc.tensor.matmul(avA[:, j0:], vA[:, t, :], e[:, 0, j0:],
                                     start=(t == 0), stop=(t == n_t - 1))
                    nc.tensor.matmul(avB[:, j0:], vB[:, t, :], e[:, 1, j0:],
                                     start=(t == 0), stop=(t == n_t - 1))
                # Evacuate AV PSUM to SBUF immediately so the next pair's AV
                # accumulation can start while the softmax denominator chain
                # (Ln/Exp/broadcast) runs against the SBUF copy.
                avSA = sbs.tile([65, TOKC], f32, tag="avS", bufs=4,
                                name=f"avSA_{c}_{p}")
                nc.vector.tensor_copy(avSA[:], avA[:])
                avSB = sbs.tile([65, TOKC], f32, tag="avS", bufs=4,
                                name=f"avSB_{c}_{p}")
                nc.vector.tensor_copy(avSB[:], avB[:])
                # softmax denominators: row 64. 1/d = exp(-ln(d)) on ACT
                # (same table set as Exp -> no table reload).
                lnd = sbs.tile([1, 2, TOKC], f32, tag="lnd", bufs=2,
                               name=f"lnd_{c}_{p}")
                nc.scalar.activation(lnd[:, 0, :], avSA[64:65, :], AF.Ln)
                nc.scalar.activation(lnd[:, 1, :], avSB[64:65, :], AF.Ln)
                invd = sbs.tile([1, 2, TOKC], f32, tag="invd", bufs=2,
                                name=f"invd_{c}_{p}")
                nc.scalar.activation(invd[:], lnd[:], AF.Exp, scale=-1.0)
                dbA = sbs.tile([64, TOKC], f32, tag="dbA", bufs=2,
                               name=f"dbA_{c}_{p}")
                nc.gpsimd.partition_broadcast(dbA[:], invd[:, 0, :], channels=64)
                dbB = sbs.tile([64, TOKC], f32, tag="dbB", bufs=2,
                               name=f"dbB_{c}_{p}")
                nc.gpsimd.partition_broadcast(dbB[:], invd[:, 1, :], channels=64)
                nc.vector.tensor_tensor(oT[0:64, p, cs], avSA[0:64, :],
                                        dbA[:], ALU.mult)
                nc.vector.tensor_tensor(oT[64:128, p, cs], avSB[0:64, :],
                                        dbB[:], ALU.mult)

            def emit_outproj(c):
                cs = slice(c * TOKC, (c + 1) * TOKC)
                for m in range(16):
                    op = ps.tile([128, TOKC], f32, tag="mm", bufs=2,
                                 name=f"op_{c}_{m}")
                    for kt in range(4):
                        nc.tensor.matmul(op[:], w_out_sb[:, kt, ts(m, 128)],
                                         oT[:, kt, cs],
                                         start=(kt == 0), stop=(kt == 3))
                    yt = sbs.tile([128, TOKC], MDT, tag="yt", bufs=2,
                                  name=f"yt_{c}_{m}")
                    nc.scalar.copy(yt[:], op[:])
                    nc.gpsimd.dma_start(yT_d[ts(m, 128), cs], yt[:])

            # Software-pipelined emission: next chunk's x load / prelude /
            # k,v projections are emitted mid-way through the current
            # chunk's pair loop so their PE/DVE work fills the exp-bound
            # attention phase, and the sync queue sees the next x DMA
            # before transposes that wait on late producers.
            load_x(0)
            emit_prelude(0)
            emit_inproj_m(0, 4)
            emit_inproj_m(0, 5)
            for c in range(NCH):
                for p in range(4):
                    emit_inproj_m(c, p)
                    if p == 0 and c + 1 < NCH:
                        load_x(c + 1)
                    emit_attn_pair(c, p)
                    if p == 1 and c + 1 < NCH:
                        emit_prelude(c + 1)
                    if p == 2 and c + 1 < NCH:
                        emit_inproj_m(c + 1, 4)
                        emit_inproj_m(c + 1, 5)
                emit_outproj(c)

    nc.finalize()
    return nc


# ------------------------------- host side ----------------------------------

def _rope_tables(S):
    inv_freq = ROPE_THETA ** (-np.arange(0, 64, 2, dtype=np.float64) / 64.0)
    ang = np.arange(S, dtype=np.float64)[:, None] * inv_freq[None, :]  # [S, 32]
    cosb = np.cos(ang).T.astype(np.float32)   # [32, S]
    sinb = np.sin(ang).T.astype(np.float32)
    cos128 = np.tile(cosb, (4, 1))                               # [128, S]
    sin128 = np.concatenate([sinb, -sinb, sinb, -sinb], axis=0)  # [128, S]
    return np.ascontiguousarray(cos128), np.ascontiguousarray(sin128)


def host_prepare(x, w_in, w_out, rms_w):
    """Build the 8 per-core input maps."""
    S = x.shape[1]
    x = np.asarray(x, dtype=np.float32)
    w_eff = np.asarray(w_in, dtype=np.float32) * np.asarray(rms_w, np.float32)[None, :]
    w_out = np.asarray(w_out, dtype=np.float32)
    cos128, sin128 = _rope_tables(S)
    tri = np.ascontiguousarray(np.triu(np.ones((128, 128), dtype=np.float32)))
    qscale = np.float32(64 ** -0.5)

    in_maps = []
    for core in range(NCORES):
        b, j = divmod(core, 4)
        g0, g1 = 2 * j, 2 * j + 1
        rows = []
        for p in range(4):
            for g in (g0, g1):
                rows.extend(range((g * 4 + p) * 64, (g * 4 + p) * 64 + 64))
        for g in (g0, g1):
            rows.extend(range(2048 + g * 64, 2048 + g * 64 + 64))
        for g in (g0, g1):
            rows.extend(range(2560 + g * 64, 2560 + g * 64 + 64))
        w_slice = w_eff[rows, :].copy()
        w_slice[:512, :] *= qscale
        cols = []
        for p in range(4):
            for g in (g0, g1):
                cols.extend(range((g * 4 + p) * 64, (g * 4 + p) * 64 + 64))
        in_maps.append({
            "xT": np.ascontiguousarray(x[b].T).astype(MDT_NP),
            "w_inT": np.ascontiguousarray(w_slice.T).astype(MDT_NP),
            "w_outT": np.ascontiguousarray(w_out[:, cols].T).astype(MDT_NP),
            "cos_t": cos128.astype(MDT_NP),
            "sin_t": sin128.astype(MDT_NP),
            "tri": tri.astype(MDT_NP),
            "oner": np.ones(1, dtype=MDT_NP),
            "epsc": np.full(1, RMS_EPS, dtype=np.float32),
        })
    return in_maps


def assemble(x, results):
    x = np.asarray(x, dtype=np.float32)
    b0 = sum(np.asarray(results[i]["yT"], dtype=np.float32) for i in range(4))
    b1 = sum(np.asarray(results[i]["yT"], dtype=np.float32) for i in range(4, 8))
    out = np.empty_like(x)
    out[0] = x[0] + b0.T
    out[1] = x[1] + b1.T
    return out


_PROGRAMS = {}


def _get_program(S):
    if S not in _PROGRAMS:
        _PROGRAMS[S] = build_program(S)
    return _PROGRAMS[S]


def run(x, w_in, w_out, rms_w, trace=False):
    from concourse.bass_utils import run_bass_kernel_spmd
    nc = _get_program(x.shape[1])
    in_maps = host_prepare(x, w_in, w_out, rms_w)
    res = run_bass_kernel_spmd(nc, in_maps, list(range(NCORES)), trace=trace)
    return assemble(x, res.results), res


def kernel(x, w_in, w_out, rms_w):
    out, _ = run(np.asarray(x), np.asarray(w_in), np.asarray(w_out),
                 np.asarray(rms_w))
    return out
